# revision 13
# baseline (speedup 1.0000x reference)
"""Trainium2 Bass kernel for nn_Caps2dMatwo (capsule conv + matwo dual routing).

Sharding: 8 cores = (batch n: 4) x (h-half: 2); each core computes a 48-row
slab of one batch element independently (halo via host padding, no collectives).

Per-core pipeline (bf16 compute, fp32 psums/stats/output):
  conv:      data-stationary patch matmuls -> X [(j4,z32), (s9, pix32, co8)]
  transform: tile_position 32x32 block matmuls -> u_hat psum [(co,tp,a,b), px]
  T2:        ACT drain -> bf16 staging, DMA-transpose -> U pixel-major
  routing:   2 iters + final on DVE/ACT (both squashes are per-t scalings)
  out:       PE transpose + scatter DMA -> [256ch, 48, 96] per core
Layouts validated against reference by golden.py (max err 7e-7 in fp32).
"""
import sys
import numpy as np

sys.path.insert(0, "/opt/trn_rl_repo")

import concourse.bass as bass
import concourse.bacc as bacc
import concourse.mybir as mybir
from concourse import tile
from concourse.bass_utils import run_bass_kernel_spmd
import ml_dtypes

BF16 = mybir.dt.float16
F32 = mybir.dt.float32
AL = mybir.AluOpType
AF = mybir.ActivationFunctionType
AX = mybir.AxisListType

T0, T1, Z, H, W, HC = 4, 8, 32, 96, 96, 48
NBLK = 36
NTCH = 4


# ----------------------------------------------------------------------------
# host-side weight/layout construction (validated by golden.py)
# ----------------------------------------------------------------------------

def _build_weights(W_conv, W_pos, W_app, b_app):
    CW = np.zeros((96, T0, 32, 8), np.float32)
    for hi in range(8):
        for wi in range(12):
            for pi in range(4):
                for pj in range(8):
                    dy, dx = hi - pi, wi - pj
                    if 0 <= dy < 5 and 0 <= dx < 5:
                        CW[hi * 12 + wi, :, pi * 8 + pj, :] = W_conv[:, dy, dx, 0, :]

    m_pos = np.stack([W_pos[i].reshape(T1, 4, 4) for i in range(T0)])
    m_app = np.stack([W_app[i].reshape(T1, 4, 4) for i in range(T0)])
    nrm = np.sqrt(np.maximum((m_pos ** 2).sum(axis=2, keepdims=True), 1e-12))
    m_pos = m_pos / nrm

    TW = np.zeros((128, T0, 2, 128), np.float32)
    for i in range(T0):
        blkp = np.zeros((32, 128), np.float32)
        blka = np.zeros((32, 128), np.float32)
        for co in range(4):
            for tp in range(2):
                t = 2 * co + tp
                for a in range(4):
                    for b in range(4):
                        m = ((co * 2 + tp) * 4 + a) * 4 + b
                        for c in range(4):
                            z = 16 * tp + 4 * a + c
                            blkp[z, m] = m_pos[i, t, c, b]
                            blka[z, m] = m_app[i, t, c, b]
        for j in range(4):
            TW[32 * j:32 * j + 32, i, 0] = blkp
            TW[32 * j:32 * j + 32, i, 1] = blka

    RW = np.zeros((128, 128), np.float32)
    for co in range(4):
        for tp in range(2):
            for a in range(4):
                m = ((co * 2 + tp) * 4 + a) * 4
                z = 16 * tp + 4 * a + 3
                for j in range(4):
                    RW[32 * j + z, m] = 1.0

    KA = np.zeros((128, T0), np.float32)
    for i in range(T0):
        for co in range(4):
            for tp in range(2):
                t = 2 * co + tp
                for a in range(4):
                    for b in range(4):
                        m = ((co * 2 + tp) * 4 + a) * 4 + b
                        KA[m, i] = b_app[i, t] * m_app[i, t, :, b].sum()
    return CW, TW, RW, KA


_PH = np.arange(NBLK) // 3
_B3 = np.arange(NBLK) % 3
_HIDX = (4 * _PH)[:, None] + np.arange(8)[None, :]
_PWJ = (4 * _B3)[:, None] + np.arange(4)[None, :]
_WIDX = (8 * _PWJ)[:, :, None] + np.arange(12)[None, None, :]


def _build_patches(pad):
    g = pad[:, :, _HIDX[:, None, :, None], _WIDX[:, :, None, :]]
    return np.ascontiguousarray(
        g.transpose(4, 5, 0, 2, 3, 1).reshape(96, T0, NBLK, 4, Z))


def _pixel_coords(hh):
    xs = np.zeros((128, NBLK, 2), np.float32)
    for b in range(NBLK):
        ph, b3 = b // 3, b % 3
        for j in range(4):
            for pi in range(4):
                for pj in range(8):
                    part = j * 32 + pi * 8 + pj
                    xs[part, b, 0] = (8 * (4 * b3 + j) + pj) / W
                    xs[part, b, 1] = (4 * ph + pi + 48 * hh) / H
    return xs


# ----------------------------------------------------------------------------
# device kernel
# ----------------------------------------------------------------------------

def _routing_chunk(nc, mpool, rpool, U, rawt, xy, s0):
    """U: ubig chunk view [128, 9, T0, 2, 128]; rawt [128, 9, T0, 128]."""
    Uf = U.rearrange("p s i pa c -> p s i (pa c)")        # [128, 9, 4, 256]

    # coord add: U[..., pos, (g, b=k)] += xy_k * raw_b0
    raw_b0 = rawt.rearrange("p s i (g b) -> p s i g b", b=4)[:, :, :, :, 0]
    tmpc = mpool.tile([128, 9, T0, 32], BF16, name="tmpc", tag="tmpc")
    for k in range(2):
        xyb = xy[:, s0:s0 + 9, k].unsqueeze(2).unsqueeze(3)
        xyb = xyb.broadcast_to([128, 9, T0, 32])
        nc.vector.tensor_tensor(tmpc[:], raw_b0, xyb, op=AL.mult)
        usl = U[:, :, :, 0].rearrange(
            "p s i (g b) -> p s i g b", b=4)[:, :, :, :, k]
        nc.vector.tensor_tensor(usl, tmpc[:], usl, op=AL.add)

    p = rpool.tile([128, 9, 2, 128], BF16, name="p", tag="p", bufs=1)
    ts1 = rpool.tile([128, 9, 256], BF16, name="ts1", tag="ts", bufs=2)
    ts2 = rpool.tile([128, 9, 256], BF16, name="ts2", tag="ts", bufs=2)
    pf = p[:].rearrange("p s pa c -> p s (pa c)")
    nc.vector.tensor_tensor(ts1[:], Uf[:, :, 0], Uf[:, :, 1], op=AL.add)
    nc.vector.tensor_tensor(ts2[:], Uf[:, :, 2], Uf[:, :, 3], op=AL.add)
    nc.vector.tensor_tensor(pf, ts1[:], ts2[:], op=AL.add)

    def stats(scale_n2, scale_a, tag):
        m = mpool.tile([128, 9, 8], BF16, name=f"m{tag}", tag="st_m")
        nc.vector.tensor_reduce(
            m[:], p[:, :, 0].rearrange("p s (t z) -> p s t z", z=16),
            axis=AX.X, op=AL.max, apply_absolute_value=True)
        sfp = mpool.tile([128, 9, 8], F32, name=f"sfp{tag}", tag=f"sfp{tag}",
                         bufs=1)
        nc.vector.reciprocal(sfp[:], m[:])
        sq = mpool.tile([128, 9, 8, 16], BF16, name=f"sq{tag}", tag="st_sq")
        papp = p[:, :, 1].rearrange("p s (t z) -> p s t z", z=16)
        nc.vector.tensor_tensor(sq[:], papp, papp, op=AL.mult)
        n2 = mpool.tile([128, 9, 8], F32, name=f"n2{tag}", tag="st_n2")
        nc.vector.tensor_reduce(n2[:], sq[:], axis=AX.X, op=AL.add)
        if scale_n2 != 1.0:
            nc.vector.tensor_scalar_mul(n2[:], n2[:], scale_n2)
        nsq = mpool.tile([128, 9, 8], F32, name=f"nsq{tag}", tag="st_nsq")
        nc.vector.tensor_scalar_add(nsq[:], n2[:], 1e-9)
        nc.scalar.sqrt(nsq[:], nsq[:])
        den = mpool.tile([128, 9, 8], F32, name=f"den{tag}", tag="st_den")
        nc.vector.tensor_scalar_add(den[:], n2[:], 1.0)
        nc.vector.tensor_tensor(den[:], den[:], nsq[:], op=AL.mult)
        rec = mpool.tile([128, 9, 8], F32, name=f"rec{tag}", tag="st_rec")
        nc.vector.reciprocal(rec[:], den[:])
        sfa = mpool.tile([128, 9, 8], F32, name=f"sfa{tag}", tag=f"sfa{tag}",
                         bufs=1)
        nc.vector.tensor_tensor(sfa[:], n2[:], rec[:], op=AL.mult)
        if scale_a != 1.0:
            nc.vector.tensor_scalar_mul(sfa[:], sfa[:], scale_a)
        return sfp, sfa

    w = rpool.tile([128, 9, T0, 256], BF16, name="w", tag="w", bufs=1)

    def araw(tag):
        """w already holds U*p'; tree-reduce z -> ar [128, 9, T0, 16] bf16."""
        wz = w[:].rearrange("p s i (g z) -> p s i g z", z=16)
        t8 = rpool.tile([128, 9, T0, 16, 8], BF16, name=f"t8{tag}", tag="t8",
                        bufs=1)
        nc.vector.tensor_tensor(t8[:], wz[:, :, :, :, 0:8], wz[:, :, :, :, 8:16],
                                op=AL.add)
        t4 = rpool.tile([128, 9, T0, 16, 4], BF16, name=f"t4{tag}", tag="t4",
                        bufs=1)
        nc.vector.tensor_tensor(t4[:], t8[:, :, :, :, 0:4], t8[:, :, :, :, 4:8],
                                op=AL.add)
        t2 = rpool.tile([128, 9, T0, 16, 2], BF16, name=f"t2{tag}", tag="t2",
                        bufs=1)
        nc.vector.tensor_tensor(t2[:], t4[:, :, :, :, 0:2], t4[:, :, :, :, 2:4],
                                op=AL.add)
        ar = rpool.tile([128, 9, T0, 16], BF16, name=f"ar{tag}", tag="ar",
                        bufs=1)
        nc.vector.tensor_tensor(ar[:], t2[:, :, :, :, 0], t2[:, :, :, :, 1],
                                op=AL.add)
        return ar

    def mult_w_by_p():
        pb = pf.unsqueeze(2).broadcast_to([128, 9, T0, 256])
        nc.vector.tensor_tensor(w[:], Uf, pb, op=AL.mult)

    def mult_w_by_r(r):
        rb = r[:].unsqueeze(4).broadcast_to([128, 9, T0, 8, 16])
        wv = w[:].rearrange("p s i (pa t z) -> p s i pa t z", pa=2, z=16)
        uv = Uf.rearrange("p s i (pa t z) -> p s i pa t z", pa=2, z=16)
        for pa in range(2):
            nc.vector.tensor_tensor(
                wv[:, :, :, pa], uv[:, :, :, pa], rb, op=AL.mult)

    def sum_w_into_p():
        wf = w[:].rearrange("p s i c -> p s (i c)")
        nc.vector.tensor_tensor(ts1[:], wf[:, :, 0:256], wf[:, :, 256:512],
                                op=AL.add)
        nc.vector.tensor_tensor(ts2[:], wf[:, :, 512:768], wf[:, :, 768:1024],
                                op=AL.add)
        nc.vector.tensor_tensor(pf, ts1[:], ts2[:], op=AL.add)

    def routstep(sfp, sfa, bacc, first, tag):
        ar = araw(tag)
        arr = ar[:].rearrange("p s i (pa t) -> p s i pa t", pa=2)
        ta = mpool.tile([128, 9, T0, 8], F32, name=f"ta{tag}", tag="rt_ta")
        tb = mpool.tile([128, 9, T0, 8], F32, name=f"tb{tag}", tag="rt_tb")
        sfpb = sfp[:].unsqueeze(2).broadcast_to([128, 9, T0, 8])
        sfab = sfa[:].unsqueeze(2).broadcast_to([128, 9, T0, 8])
        nc.vector.tensor_tensor(ta[:], arr[:, :, :, 0], sfpb, op=AL.mult)
        nc.vector.tensor_tensor(tb[:], arr[:, :, :, 1], sfab, op=AL.mult)
        if first:
            nc.vector.tensor_tensor(bacc[:], ta[:], tb[:], op=AL.mult)
        else:
            nc.vector.tensor_tensor(ta[:], ta[:], tb[:], op=AL.mult)
            nc.vector.tensor_tensor(bacc[:], bacc[:], ta[:], op=AL.add)

    # iter 1 (r = 0.5 folded into scalings)
    sfp1, sfa1 = stats(0.25, 0.5, "1")
    mult_w_by_p()
    bacc = rpool.tile([128, 9, T0, 8], F32, name="bacc", tag="bacc", bufs=1)
    routstep(sfp1, sfa1, bacc, True, "r1")

    # iter 2
    r2 = rpool.tile([128, 9, T0, 8], BF16, name="r2", tag="r2", bufs=1)
    nc.scalar.activation(r2[:], bacc[:], AF.Sigmoid)
    mult_w_by_r(r2)
    sum_w_into_p()
    sfp2, sfa2 = stats(1.0, 1.0, "2")
    mult_w_by_p()
    routstep(sfp2, sfa2, bacc, False, "r2")

    # final
    cR = rpool.tile([128, 9, T0, 8], BF16, name="cR", tag="r2", bufs=1)
    nc.scalar.activation(cR[:], bacc[:], AF.Sigmoid)
    mult_w_by_r(cR)
    sum_w_into_p()
    sfp3, sfa3 = stats(1.0, 1.0, "3")
    v3 = rpool.tile([128, 9, 2, 128], BF16, name="v3", tag="v3", bufs=1)
    sfp3b = sfp3[:].unsqueeze(3).broadcast_to([128, 9, 8, 16])
    sfa3b = sfa3[:].unsqueeze(3).broadcast_to([128, 9, 8, 16])
    nc.vector.tensor_tensor(
        v3[:, :, 0].rearrange("p s (t z) -> p s t z", z=16),
        p[:, :, 0].rearrange("p s (t z) -> p s t z", z=16), sfp3b, op=AL.mult)
    nc.vector.tensor_tensor(
        v3[:, :, 1].rearrange("p s (t z) -> p s t z", z=16),
        p[:, :, 1].rearrange("p s (t z) -> p s t z", z=16), sfa3b, op=AL.mult)
    return v3


def _build_nc():
    nc = bacc.Bacc(None)
    P_d = nc.dram_tensor("patches", [96, T0, NBLK, 4, Z], BF16, kind="ExternalInput")
    CW_d = nc.dram_tensor("convw", [96, T0, 32, 8], BF16, kind="ExternalInput")
    TW_d = nc.dram_tensor("tw", [128, T0, 2, 128], BF16, kind="ExternalInput")
    RW_d = nc.dram_tensor("rw", [128, 128], BF16, kind="ExternalInput")
    KA_d = nc.dram_tensor("ka", [128, T0], F32, kind="ExternalInput")
    XY_d = nc.dram_tensor("xy", [128, NBLK, 2], F32, kind="ExternalInput")
    ID_d = nc.dram_tensor("ident", [128, 128], BF16, kind="ExternalInput")
    OUT_d = nc.dram_tensor("out", [256, NBLK, 128], F32, kind="ExternalOutput")

    with tile.TileContext(nc) as tc:
        with (
            tc.tile_pool(name="const", bufs=1) as cpool,
            tc.tile_pool(name="pload", bufs=2) as ppool,
            tc.tile_pool(name="xbuf", bufs=2) as xpool,
            tc.tile_pool(name="stage", bufs=2) as spool,
            tc.tile_pool(name="ubig", bufs=1) as upool,
            tc.tile_pool(name="rscr", bufs=1) as rpool,
            tc.tile_pool(name="small", bufs=2) as mpool,
            tc.tile_pool(name="ps_x", bufs=2, space="PSUM") as psx,
            tc.tile_pool(name="ps_uh", bufs=4, space="PSUM") as psuh,
        ):
            cw = cpool.tile([96, T0, 32, 8], BF16, name="cw")
            nc.sync.dma_start(cw[:], CW_d[:])
            tw = cpool.tile([128, T0, 2, 128], BF16, name="tw")
            nc.sync.dma_start(tw[:], TW_d[:])
            rwt = cpool.tile([128, 128], BF16, name="rwt")
            nc.sync.dma_start(rwt[:], RW_d[:])
            ka = cpool.tile([128, T0], F32, name="ka")
            nc.sync.dma_start(ka[:], KA_d[:])
            xy = cpool.tile([128, NBLK, 2], F32, name="xy")
            nc.sync.dma_start(xy[:], XY_d[:])
            ident = cpool.tile([128, 128], BF16, name="ident")
            nc.sync.dma_start(ident[:], ID_d[:])

            ubig = upool.tile([128, NBLK, T0, 2, 128], BF16, name="ubig")

            for tch in range(NTCH):
                s0 = tch * 9
                rawt = spool.tile([128, 9, T0, 128], BF16, name="rawt",
                                  tag="rawt", bufs=2)
                for i in range(T0):
                    pt = ppool.tile([96, 9, 4, Z], BF16, name="pt", tag="pt")
                    nc.sync.dma_start(pt[:], P_d[:, i, s0:s0 + 9])
                    # conv: 9 data-stationary matmuls, F = (pix32, co8) = 256
                    xsb = xpool.tile([128, 9, 32, 8], BF16, name="xsb", tag="xsb")
                    for s in range(9):
                        xps = psx.tile([128, 256], F32, name="xps", tag="xps")
                        nc.tensor.matmul(
                            xps[:],
                            pt[:, s].rearrange("p j z -> p (j z)"),
                            cw[:, i].rearrange("p f c -> p (f c)"),
                            start=True, stop=True)
                        nc.scalar.copy(
                            xsb[:, s].rearrange("p f c -> p (f c)"), xps[:])
                    # transform pos/app: per rp one psum tile [128, (s9, pix32)]
                    stp = spool.tile([128, 9, 4, 32], BF16, name="stp", tag="stp")
                    sta = spool.tile([128, 9, 4, 32], BF16, name="sta", tag="sta")
                    for pa, stg_t in ((0, stp), (1, sta)):
                        for rp in range(4):
                            ups = psuh.tile([128, 9, 32], F32, name="ups", tag="uh")
                            for cp in range(4):
                                nc.tensor.matmul(
                                    ups[32 * cp:32 * cp + 32],
                                    tw[32 * rp:32 * rp + 32, i, pa,
                                       32 * cp:32 * cp + 32],
                                    xsb[32 * rp:32 * rp + 32, :, :, pa * 4 + cp],
                                    start=True, stop=True,
                                    tile_position=(32 * rp, 32 * cp))
                            nc.scalar.copy(stg_t[:, :, rp, :], ups[:])
                    # raw extraction (pos conv channels, c=3 picks)
                    rstg = spool.tile([128, 9, 4, 32], BF16, name="rstg", tag="rstg")
                    for rp in range(4):
                        rps = psuh.tile([128, 9, 32], F32, name="rps", tag="uh")
                        for cp in range(4):
                            nc.tensor.matmul(
                                rps[32 * cp:32 * cp + 32],
                                rwt[32 * rp:32 * rp + 32, 32 * cp:32 * cp + 32],
                                xsb[32 * rp:32 * rp + 32, :, :, cp],
                                start=True, stop=True,
                                tile_position=(32 * rp, 32 * cp))
                        nc.scalar.copy(rstg[:, :, rp, :], rps[:])
                    # bias add on app staging (per-partition scalar)
                    nc.vector.tensor_scalar_add(
                        sta[:].rearrange("p a b c -> p (a b c)"),
                        sta[:].rearrange("p a b c -> p (a b c)"),
                        ka[:, i:i + 1])
                    # DMA-transpose to pixel-major layouts
                    for s in range(9):
                        nc.sync.dma_start(
                            ubig[:, s0 + s, i, 0, :],
                            stp[:, s].rearrange("p j f -> p (j f)"),
                            transpose=True)
                        nc.sync.dma_start(
                            ubig[:, s0 + s, i, 1, :],
                            sta[:, s].rearrange("p j f -> p (j f)"),
                            transpose=True)
                        nc.sync.dma_start(
                            rawt[:, s, i, :],
                            rstg[:, s].rearrange("p j f -> p (j f)"),
                            transpose=True)

                # routing for this chunk
                U = ubig[:, s0:s0 + 9]
                v3 = _routing_chunk(nc, mpool, rpool, U, rawt[:], xy, s0)

                # output: PE transpose + scatter DMA
                for s in range(9):
                    blk = s0 + s
                    for pa in range(2):
                        ops = psx.tile([128, 128], BF16, name="ops", tag="xps")
                        nc.tensor.transpose(ops[:], v3[:, s, pa], ident[:])
                        osb = mpool.tile([128, 128], F32, name="osb", tag="osb")
                        nc.scalar.copy(osb[:], ops[:])
                        nc.sync.dma_start(
                            OUT_d[128 * pa:128 * pa + 128, blk, :], osb[:])
    nc.finalize()
    return nc


_NC_CACHE = None


def _get_nc():
    global _NC_CACHE
    if _NC_CACHE is None:
        _NC_CACHE = _build_nc()
    return _NC_CACHE


def kernel(input_tensor, W_conv, W_pos, W_app, b_app):
    input_tensor = np.asarray(input_tensor, np.float32)
    CW, TW, RW, KA = _build_weights(np.asarray(W_conv, np.float32),
                                    np.asarray(W_pos, np.float32),
                                    np.asarray(W_app, np.float32),
                                    np.asarray(b_app, np.float32))
    N = input_tensor.shape[0]
    full_pad = np.pad(input_tensor, ((0, 0), (0, 0), (0, 0), (2, 2), (2, 2)))
    bf = np.float16
    ident = np.eye(128, dtype=np.float32)
    in_maps = []
    for c in range(8):
        n, hh = c // 2, c % 2
        sl = full_pad[n, :, :, 48 * hh:48 * hh + 52, :]
        in_maps.append({
            "patches": _build_patches(sl).astype(bf),
            "convw": CW.astype(bf),
            "tw": TW.astype(bf),
            "rw": RW.astype(bf),
            "ka": KA.astype(np.float32),
            "xy": _pixel_coords(hh).astype(np.float32),
            "ident": ident.astype(bf),
        })
    nc = _get_nc()
    kres = run_bass_kernel_spmd(nc, in_maps, core_ids=list(range(8)))
    global LAST_RESULT
    LAST_RESULT = kres
    res = kres.results
    # unscramble: out dram [256=(pa,co,tp,z16), blk36, px128=(j,pi,pj)]
    blk = np.arange(NBLK)
    j = np.arange(4)
    pi = np.arange(4)
    pj = np.arange(8)
    hmap = (4 * (blk // 3))[:, None, None, None] + pi[None, None, :, None]
    hmap = np.broadcast_to(hmap, (NBLK, 4, 4, 8)).ravel()
    wmap = (32 * (blk % 3))[:, None, None, None] + 8 * j[None, :, None, None] \
        + pj[None, None, None, :]
    wmap = np.broadcast_to(wmap, (NBLK, 4, 4, 8)).ravel()
    out = np.zeros((N, T1, Z, H, W), np.float32)
    for c in range(8):
        n, hh = c // 2, c % 2
        img = np.zeros((256, HC, W), np.float32)
        img[:, hmap, wmap] = res[c]["out"].reshape(256, NBLK * 128)
        o = img.reshape(2, 4, 2, 16, HC, W)
        for pa in range(2):
            for co in range(4):
                for tp in range(2):
                    out[n, 2 * co + tp, pa * 16:pa * 16 + 16,
                        48 * hh:48 * hh + 48] = o[pa, co, tp]
    return out


# revision 15
# speedup vs baseline: 1.9416x; 1.9416x over previous
"""Trainium2 Bass kernel for nn_Caps2dMatwo (capsule conv + matwo dual routing).

Sharding: 8 cores = (batch n: 4) x (h-half: 2); each core computes a 48-row
slab of one batch element independently (halo via host padding, no collectives).

Per-core pipeline (bf16 compute, fp32 psums/stats/output):
  conv:      data-stationary patch matmuls -> X [(j4,z32), (s9, pix32, co8)]
  transform: tile_position 32x32 block matmuls -> u_hat psum [(co,tp,a,b), px]
  T2:        ACT drain -> bf16 staging, DMA-transpose -> U pixel-major
  routing:   2 iters + final on DVE/ACT (both squashes are per-t scalings)
  out:       PE transpose + scatter DMA -> [256ch, 48, 96] per core
Layouts validated against reference by golden.py (max err 7e-7 in fp32).
"""
import sys
import numpy as np

sys.path.insert(0, "/opt/trn_rl_repo")

import concourse.bass as bass
import concourse.bacc as bacc
import concourse.mybir as mybir
from concourse import tile
from concourse.bass_utils import run_bass_kernel_spmd
import ml_dtypes

BF16 = mybir.dt.float16
F32 = mybir.dt.float32
AL = mybir.AluOpType
AF = mybir.ActivationFunctionType
AX = mybir.AxisListType

T0, T1, Z, H, W, HC = 4, 8, 32, 96, 96, 48
NBLK = 36
NTCH = 4


# ----------------------------------------------------------------------------
# host-side weight/layout construction (validated by golden.py)
# ----------------------------------------------------------------------------

def _build_weights(W_conv, W_pos, W_app, b_app):
    CW = np.zeros((96, T0, 32, 8), np.float32)
    for hi in range(8):
        for wi in range(12):
            for pi in range(4):
                for pj in range(8):
                    dy, dx = hi - pi, wi - pj
                    if 0 <= dy < 5 and 0 <= dx < 5:
                        CW[hi * 12 + wi, :, pi * 8 + pj, :] = W_conv[:, dy, dx, 0, :]

    m_pos = np.stack([W_pos[i].reshape(T1, 4, 4) for i in range(T0)])
    m_app = np.stack([W_app[i].reshape(T1, 4, 4) for i in range(T0)])
    nrm = np.sqrt(np.maximum((m_pos ** 2).sum(axis=2, keepdims=True), 1e-12))
    m_pos = m_pos / nrm

    TW = np.zeros((128, T0, 2, 128), np.float32)
    for i in range(T0):
        blkp = np.zeros((32, 128), np.float32)
        blka = np.zeros((32, 128), np.float32)
        for co in range(4):
            for tp in range(2):
                t = 2 * co + tp
                for a in range(4):
                    for b in range(4):
                        m = ((co * 2 + tp) * 4 + a) * 4 + b
                        for c in range(4):
                            z = 16 * tp + 4 * a + c
                            blkp[z, m] = m_pos[i, t, c, b]
                            blka[z, m] = m_app[i, t, c, b]
        for j in range(4):
            TW[32 * j:32 * j + 32, i, 0] = blkp
            TW[32 * j:32 * j + 32, i, 1] = blka

    RW = np.zeros((128, 128), np.float32)
    for co in range(4):
        for tp in range(2):
            for a in range(4):
                m = ((co * 2 + tp) * 4 + a) * 4
                z = 16 * tp + 4 * a + 3
                for j in range(4):
                    RW[32 * j + z, m] = 1.0

    KA = np.zeros((128, T0), np.float32)
    for i in range(T0):
        for co in range(4):
            for tp in range(2):
                t = 2 * co + tp
                for a in range(4):
                    for b in range(4):
                        m = ((co * 2 + tp) * 4 + a) * 4 + b
                        KA[m, i] = b_app[i, t] * m_app[i, t, :, b].sum()
    return CW, TW, RW, KA


_PH = np.arange(NBLK) // 3
_B3 = np.arange(NBLK) % 3
_HIDX = (4 * _PH)[:, None] + np.arange(8)[None, :]
_PWJ = (4 * _B3)[:, None] + np.arange(4)[None, :]
_WIDX = (8 * _PWJ)[:, :, None] + np.arange(12)[None, None, :]


def _build_patches(pad):
    g = pad[:, :, _HIDX[:, None, :, None], _WIDX[:, :, None, :]]
    return np.ascontiguousarray(
        g.transpose(4, 5, 0, 2, 3, 1).reshape(96, T0, NBLK, 4, Z))


def _pixel_coords(hh):
    xs = np.zeros((128, NBLK, 2), np.float32)
    for b in range(NBLK):
        ph, b3 = b // 3, b % 3
        for j in range(4):
            for pi in range(4):
                for pj in range(8):
                    part = j * 32 + pi * 8 + pj
                    xs[part, b, 0] = (8 * (4 * b3 + j) + pj) / W
                    xs[part, b, 1] = (4 * ph + pi + 48 * hh) / H
    return xs


# ----------------------------------------------------------------------------
# device kernel
# ----------------------------------------------------------------------------

def _routing_chunk(nc, mpool, rpool, U, rawt, xy, s0):
    """U: ubig chunk view [128, 9, T0, 2, 128]; rawt [128, 9, T0, 128]."""
    Uf = U.rearrange("p s i pa c -> p s i (pa c)")        # [128, 9, 4, 256]

    # coord add: U[..., pos, (g, b=k)] += xy_k * raw_b0
    raw_b0 = rawt.rearrange("p s i (g b) -> p s i g b", b=4)[:, :, :, :, 0]
    tmpc = mpool.tile([128, 9, T0, 32], BF16, name="tmpc", tag="tmpc")
    for k in range(2):
        xyb = xy[:, s0:s0 + 9, k].unsqueeze(2).unsqueeze(3)
        xyb = xyb.broadcast_to([128, 9, T0, 32])
        nc.vector.tensor_tensor(tmpc[:], raw_b0, xyb, op=AL.mult)
        usl = U[:, :, :, 0].rearrange(
            "p s i (g b) -> p s i g b", b=4)[:, :, :, :, k]
        nc.vector.tensor_tensor(usl, tmpc[:], usl, op=AL.add)

    p = rpool.tile([128, 9, 2, 128], BF16, name="p", tag="p", bufs=1)
    ts1 = rpool.tile([128, 9, 256], BF16, name="ts1", tag="ts", bufs=2)
    ts2 = rpool.tile([128, 9, 256], BF16, name="ts2", tag="ts", bufs=2)
    pf = p[:].rearrange("p s pa c -> p s (pa c)")
    nc.vector.tensor_tensor(ts1[:], Uf[:, :, 0], Uf[:, :, 1], op=AL.add)
    nc.vector.tensor_tensor(ts2[:], Uf[:, :, 2], Uf[:, :, 3], op=AL.add)
    nc.vector.tensor_tensor(pf, ts1[:], ts2[:], op=AL.add)

    def stats(scale_n2, scale_a, tag):
        m = mpool.tile([128, 9, 8], BF16, name=f"m{tag}", tag="st_m")
        nc.vector.tensor_reduce(
            m[:], p[:, :, 0].rearrange("p s (t z) -> p s t z", z=16),
            axis=AX.X, op=AL.max, apply_absolute_value=True)
        sfp = mpool.tile([128, 9, 8], F32, name=f"sfp{tag}", tag=f"sfp{tag}",
                         bufs=1)
        nc.vector.reciprocal(sfp[:], m[:])
        sq = mpool.tile([128, 9, 8, 16], BF16, name=f"sq{tag}", tag="st_sq")
        papp = p[:, :, 1].rearrange("p s (t z) -> p s t z", z=16)
        nc.vector.tensor_tensor(sq[:], papp, papp, op=AL.mult)
        n2 = mpool.tile([128, 9, 8], F32, name=f"n2{tag}", tag="st_n2")
        nc.vector.tensor_reduce(n2[:], sq[:], axis=AX.X, op=AL.add)
        if scale_n2 != 1.0:
            nc.vector.tensor_scalar_mul(n2[:], n2[:], scale_n2)
        nsq = mpool.tile([128, 9, 8], F32, name=f"nsq{tag}", tag="st_nsq")
        nc.vector.tensor_scalar_add(nsq[:], n2[:], 1e-9)
        nc.scalar.sqrt(nsq[:], nsq[:])
        den = mpool.tile([128, 9, 8], F32, name=f"den{tag}", tag="st_den")
        nc.vector.tensor_scalar_add(den[:], n2[:], 1.0)
        nc.vector.tensor_tensor(den[:], den[:], nsq[:], op=AL.mult)
        rec = mpool.tile([128, 9, 8], F32, name=f"rec{tag}", tag="st_rec")
        nc.vector.reciprocal(rec[:], den[:])
        sfa = mpool.tile([128, 9, 8], F32, name=f"sfa{tag}", tag=f"sfa{tag}",
                         bufs=1)
        nc.vector.tensor_tensor(sfa[:], n2[:], rec[:], op=AL.mult)
        if scale_a != 1.0:
            nc.vector.tensor_scalar_mul(sfa[:], sfa[:], scale_a)
        return sfp, sfa

    w = rpool.tile([128, 9, T0, 256], BF16, name="w", tag="w", bufs=1)

    def araw(tag):
        """w already holds U*p'; tree-reduce z -> ar [128, 9, T0, 16] bf16."""
        wz = w[:].rearrange("p s i (g z) -> p s i g z", z=16)
        t8 = rpool.tile([128, 9, T0, 16, 8], BF16, name=f"t8{tag}", tag="t8",
                        bufs=1)
        nc.vector.tensor_tensor(t8[:], wz[:, :, :, :, 0:8], wz[:, :, :, :, 8:16],
                                op=AL.add)
        t4 = rpool.tile([128, 9, T0, 16, 4], BF16, name=f"t4{tag}", tag="t4",
                        bufs=1)
        nc.vector.tensor_tensor(t4[:], t8[:, :, :, :, 0:4], t8[:, :, :, :, 4:8],
                                op=AL.add)
        t2 = rpool.tile([128, 9, T0, 16, 2], BF16, name=f"t2{tag}", tag="t2",
                        bufs=1)
        nc.vector.tensor_tensor(t2[:], t4[:, :, :, :, 0:2], t4[:, :, :, :, 2:4],
                                op=AL.add)
        ar = rpool.tile([128, 9, T0, 16], BF16, name=f"ar{tag}", tag="ar",
                        bufs=1)
        nc.vector.tensor_tensor(ar[:], t2[:, :, :, :, 0], t2[:, :, :, :, 1],
                                op=AL.add)
        return ar

    def mult_w_by_p():
        pb = pf.unsqueeze(2).broadcast_to([128, 9, T0, 256])
        nc.vector.tensor_tensor(w[:], Uf, pb, op=AL.mult)

    def mult_w_by_r(r):
        rb = r[:].unsqueeze(4).broadcast_to([128, 9, T0, 8, 16])
        wv = w[:].rearrange("p s i (pa t z) -> p s i pa t z", pa=2, z=16)
        uv = Uf.rearrange("p s i (pa t z) -> p s i pa t z", pa=2, z=16)
        for pa in range(2):
            nc.vector.tensor_tensor(
                wv[:, :, :, pa], uv[:, :, :, pa], rb, op=AL.mult)

    def sum_w_into_p():
        wf = w[:].rearrange("p s i c -> p s (i c)")
        nc.vector.tensor_tensor(ts1[:], wf[:, :, 0:256], wf[:, :, 256:512],
                                op=AL.add)
        nc.vector.tensor_tensor(ts2[:], wf[:, :, 512:768], wf[:, :, 768:1024],
                                op=AL.add)
        nc.vector.tensor_tensor(pf, ts1[:], ts2[:], op=AL.add)

    def routstep(sfp, sfa, bacc, first, tag):
        ar = araw(tag)
        arr = ar[:].rearrange("p s i (pa t) -> p s i pa t", pa=2)
        ta = mpool.tile([128, 9, T0, 8], F32, name=f"ta{tag}", tag="rt_ta")
        tb = mpool.tile([128, 9, T0, 8], F32, name=f"tb{tag}", tag="rt_tb")
        sfpb = sfp[:].unsqueeze(2).broadcast_to([128, 9, T0, 8])
        sfab = sfa[:].unsqueeze(2).broadcast_to([128, 9, T0, 8])
        nc.vector.tensor_tensor(ta[:], arr[:, :, :, 0], sfpb, op=AL.mult)
        nc.vector.tensor_tensor(tb[:], arr[:, :, :, 1], sfab, op=AL.mult)
        if first:
            nc.vector.tensor_tensor(bacc[:], ta[:], tb[:], op=AL.mult)
        else:
            nc.vector.tensor_tensor(ta[:], ta[:], tb[:], op=AL.mult)
            nc.vector.tensor_tensor(bacc[:], bacc[:], ta[:], op=AL.add)

    # iter 1 (r = 0.5 folded into scalings)
    sfp1, sfa1 = stats(0.25, 0.5, "1")
    mult_w_by_p()
    bacc = rpool.tile([128, 9, T0, 8], F32, name="bacc", tag="bacc", bufs=1)
    routstep(sfp1, sfa1, bacc, True, "r1")

    # iter 2
    r2 = rpool.tile([128, 9, T0, 8], BF16, name="r2", tag="r2", bufs=1)
    nc.scalar.activation(r2[:], bacc[:], AF.Sigmoid)
    mult_w_by_r(r2)
    sum_w_into_p()
    sfp2, sfa2 = stats(1.0, 1.0, "2")
    mult_w_by_p()
    routstep(sfp2, sfa2, bacc, False, "r2")

    # final
    cR = rpool.tile([128, 9, T0, 8], BF16, name="cR", tag="r2", bufs=1)
    nc.scalar.activation(cR[:], bacc[:], AF.Sigmoid)
    mult_w_by_r(cR)
    sum_w_into_p()
    sfp3, sfa3 = stats(1.0, 1.0, "3")
    v3 = rpool.tile([128, 9, 2, 128], BF16, name="v3", tag="v3", bufs=1)
    sfp3b = sfp3[:].unsqueeze(3).broadcast_to([128, 9, 8, 16])
    sfa3b = sfa3[:].unsqueeze(3).broadcast_to([128, 9, 8, 16])
    nc.vector.tensor_tensor(
        v3[:, :, 0].rearrange("p s (t z) -> p s t z", z=16),
        p[:, :, 0].rearrange("p s (t z) -> p s t z", z=16), sfp3b, op=AL.mult)
    nc.vector.tensor_tensor(
        v3[:, :, 1].rearrange("p s (t z) -> p s t z", z=16),
        p[:, :, 1].rearrange("p s (t z) -> p s t z", z=16), sfa3b, op=AL.mult)
    return v3


def _build_nc():
    nc = bacc.Bacc(None)
    P_d = nc.dram_tensor("patches", [96, T0, NBLK, 4, Z], BF16, kind="ExternalInput")
    CW_d = nc.dram_tensor("convw", [96, T0, 32, 8], BF16, kind="ExternalInput")
    TW_d = nc.dram_tensor("tw", [128, T0, 2, 128], BF16, kind="ExternalInput")
    RW_d = nc.dram_tensor("rw", [128, 128], BF16, kind="ExternalInput")
    KA_d = nc.dram_tensor("ka", [128, T0], F32, kind="ExternalInput")
    XY_d = nc.dram_tensor("xy", [128, NBLK, 2], F32, kind="ExternalInput")
    ID_d = nc.dram_tensor("ident", [128, 128], BF16, kind="ExternalInput")
    OUT_d = nc.dram_tensor("out", [256, NBLK, 128], F32, kind="ExternalOutput")

    with tile.TileContext(nc) as tc:
        with (
            tc.tile_pool(name="const", bufs=1) as cpool,
            tc.tile_pool(name="pload", bufs=2) as ppool,
            tc.tile_pool(name="xbuf", bufs=2) as xpool,
            tc.tile_pool(name="stage", bufs=2) as spool,
            tc.tile_pool(name="ubig", bufs=1) as upool,
            tc.tile_pool(name="rscr", bufs=1) as rpool,
            tc.tile_pool(name="small", bufs=2) as mpool,
            tc.tile_pool(name="ps_x", bufs=2, space="PSUM") as psx,
            tc.tile_pool(name="ps_uh", bufs=4, space="PSUM") as psuh,
        ):
            cw = cpool.tile([96, T0, 32, 8], BF16, name="cw")
            nc.sync.dma_start(cw[:], CW_d[:])
            tw = cpool.tile([128, T0, 2, 128], BF16, name="tw")
            nc.sync.dma_start(tw[:], TW_d[:])
            rwt = cpool.tile([128, 128], BF16, name="rwt")
            nc.sync.dma_start(rwt[:], RW_d[:])
            ka = cpool.tile([128, T0], F32, name="ka")
            nc.sync.dma_start(ka[:], KA_d[:])
            xy = cpool.tile([128, NBLK, 2], F32, name="xy")
            nc.sync.dma_start(xy[:], XY_d[:])
            ident = cpool.tile([128, 128], BF16, name="ident")
            nc.sync.dma_start(ident[:], ID_d[:])

            ubig = upool.tile([128, NBLK, T0, 2, 128], BF16, name="ubig")

            for tch in range(NTCH):
                s0 = tch * 9
                rawt = spool.tile([128, 9, T0, 128], BF16, name="rawt",
                                  tag="rawt", bufs=1)
                for i in range(T0):
                    pt = ppool.tile([96, 9, 4, Z], BF16, name="pt", tag="pt")
                    nc.sync.dma_start(pt[:], P_d[:, i, s0:s0 + 9])
                    # conv: 9 data-stationary matmuls, F = (pix32, co8) = 256
                    xsb = xpool.tile([128, 9, 32, 8], BF16, name="xsb", tag="xsb")
                    for s in range(9):
                        xps = psx.tile([128, 256], F32, name="xps", tag="xps")
                        nc.tensor.matmul(
                            xps[:],
                            pt[:, s].rearrange("p j z -> p (j z)"),
                            cw[:, i].rearrange("p f c -> p (f c)"),
                            start=True, stop=True)
                        nc.scalar.copy(
                            xsb[:, s].rearrange("p f c -> p (f c)"), xps[:])
                    # transform pos/app: per rp one psum tile [128, (s9, pix32)]
                    stp = spool.tile([128, 9, 4, 32], BF16, name="stp", tag="stp")
                    sta = spool.tile([128, 9, 4, 32], BF16, name="sta", tag="sta")
                    for pa, stg_t in ((0, stp), (1, sta)):
                        for rp in range(4):
                            ups = psuh.tile([128, 9, 32], F32, name="ups", tag="uh")
                            for cp in range(4):
                                nc.tensor.matmul(
                                    ups[32 * cp:32 * cp + 32],
                                    tw[32 * rp:32 * rp + 32, i, pa,
                                       32 * cp:32 * cp + 32],
                                    xsb[32 * rp:32 * rp + 32, :, :, pa * 4 + cp],
                                    start=True, stop=True,
                                    tile_position=(32 * rp, 32 * cp))
                            nc.scalar.copy(stg_t[:, :, rp, :], ups[:])
                    # raw extraction (pos conv channels, c=3 picks)
                    rstg = spool.tile([128, 9, 4, 32], BF16, name="rstg", tag="rstg")
                    for rp in range(4):
                        rps = psuh.tile([128, 9, 32], F32, name="rps", tag="uh")
                        for cp in range(4):
                            nc.tensor.matmul(
                                rps[32 * cp:32 * cp + 32],
                                rwt[32 * rp:32 * rp + 32, 32 * cp:32 * cp + 32],
                                xsb[32 * rp:32 * rp + 32, :, :, cp],
                                start=True, stop=True,
                                tile_position=(32 * rp, 32 * cp))
                        nc.scalar.copy(rstg[:, :, rp, :], rps[:])
                    # bias add on app staging (per-partition scalar)
                    nc.vector.tensor_scalar_add(
                        sta[:].rearrange("p a b c -> p (a b c)"),
                        sta[:].rearrange("p a b c -> p (a b c)"),
                        ka[:, i:i + 1])
                    # batched DMA-transposes to pixel-major layouts
                    nc.sync.dma_start(
                        ubig[:, s0:s0 + 9, i, 0, :],
                        stp[:].rearrange("p s j f -> p (s j f)"), transpose=True)
                    nc.sync.dma_start(
                        ubig[:, s0:s0 + 9, i, 1, :],
                        sta[:].rearrange("p s j f -> p (s j f)"), transpose=True)
                    nc.sync.dma_start(
                        rawt[:, :, i, :],
                        rstg[:].rearrange("p s j f -> p (s j f)"), transpose=True)

                # routing for this chunk
                U = ubig[:, s0:s0 + 9]
                v3 = _routing_chunk(nc, mpool, rpool, U, rawt[:], xy, s0)

                # output: PE transpose + one batched DMA per chunk
                osb = mpool.tile([128, 2, 9, 128], F32, name="osb", tag="osb", bufs=1)
                for s in range(9):
                    for pa in range(2):
                        ops = psx.tile([128, 128], BF16, name="ops", tag="xps")
                        nc.tensor.transpose(ops[:], v3[:, s, pa], ident[:])
                        nc.scalar.copy(osb[:, pa, s, :], ops[:])
                nc.sync.dma_start(
                    OUT_d.rearrange("(pa c) b f -> c pa b f", pa=2)[:, :, s0:s0 + 9],
                    osb[:])
    nc.finalize()
    return nc


_NC_CACHE = None


def _get_nc():
    global _NC_CACHE
    if _NC_CACHE is None:
        _NC_CACHE = _build_nc()
    return _NC_CACHE


def kernel(input_tensor, W_conv, W_pos, W_app, b_app):
    input_tensor = np.asarray(input_tensor, np.float32)
    CW, TW, RW, KA = _build_weights(np.asarray(W_conv, np.float32),
                                    np.asarray(W_pos, np.float32),
                                    np.asarray(W_app, np.float32),
                                    np.asarray(b_app, np.float32))
    N = input_tensor.shape[0]
    full_pad = np.pad(input_tensor, ((0, 0), (0, 0), (0, 0), (2, 2), (2, 2)))
    bf = np.float16
    ident = np.eye(128, dtype=np.float32)
    in_maps = []
    for c in range(8):
        n, hh = c // 2, c % 2
        sl = full_pad[n, :, :, 48 * hh:48 * hh + 52, :]
        in_maps.append({
            "patches": _build_patches(sl).astype(bf),
            "convw": CW.astype(bf),
            "tw": TW.astype(bf),
            "rw": RW.astype(bf),
            "ka": KA.astype(np.float32),
            "xy": _pixel_coords(hh).astype(np.float32),
            "ident": ident.astype(bf),
        })
    nc = _get_nc()
    kres = run_bass_kernel_spmd(nc, in_maps, core_ids=list(range(8)))
    global LAST_RESULT
    LAST_RESULT = kres
    res = kres.results
    # unscramble: out dram [256=(pa,co,tp,z16), blk36, px128=(j,pi,pj)]
    blk = np.arange(NBLK)
    j = np.arange(4)
    pi = np.arange(4)
    pj = np.arange(8)
    hmap = (4 * (blk // 3))[:, None, None, None] + pi[None, None, :, None]
    hmap = np.broadcast_to(hmap, (NBLK, 4, 4, 8)).ravel()
    wmap = (32 * (blk % 3))[:, None, None, None] + 8 * j[None, :, None, None] \
        + pj[None, None, None, :]
    wmap = np.broadcast_to(wmap, (NBLK, 4, 4, 8)).ravel()
    out = np.zeros((N, T1, Z, H, W), np.float32)
    for c in range(8):
        n, hh = c // 2, c % 2
        img = np.zeros((256, HC, W), np.float32)
        img[:, hmap, wmap] = res[c]["out"].reshape(256, NBLK * 128)
        o = img.reshape(2, 4, 2, 16, HC, W)
        for pa in range(2):
            for co in range(4):
                for tp in range(2):
                    out[n, 2 * co + tp, pa * 16:pa * 16 + 16,
                        48 * hh:48 * hh + 48] = o[pa, co, tp]
    return out
